# revision 62
# baseline (speedup 1.0000x reference)
"""Distributed TRN2 Bass kernel for a dynamic-int8-quantized transformer encoder
layer — head-sharded (tensor-parallel) variant, v3.

Sharding: core c owns batch b=c//4 and heads 4g..4g+3 (g=c%4), i.e. H-dims
[512g, 512g+512).  The full batch x is replicated to every core (input
layout), so each core quantizes its batch's x locally — no x AllGather.  Each
core quantizes its own Wq/Wk/Wv column slice (no weight AllGather); attention
is fully head-local.  Before the output projection a masked 8-rank AllToAll
(cross-batch shards scaled to exact zeros via a one-hot bmask input) returns
to token sharding; the receiver merges mirror shard pairs with one bf16 add.
Wp is quantized in 8 shards and AllGathered early.

Numerics identical to the validated baseline:
  - int8 fake-quant matmuls as integer-valued bf16 matmuls (fp32 PSUM).
  - softmax without max-subtraction; two-pass with global amax(probs) =
    max_i M_i/S_i; pass 2 recomputes QK^T and adds c'_i = -ln(S_i)/s_qk
    (bf16 hi+mid+lo split via a k=3 matmul), bias B = -ln(s_attn) in the exp.
  - round-to-nearest-even: x + 1.5*2^23 - 1.5*2^23 in fp32.
"""

import math
import os
import sys
from contextlib import ExitStack

import numpy as np

sys.path.insert(0, "/opt/trn_rl_repo")

B = 2
S = 2048
H = 2048
NH = 16
D = 128
NC = 8
GROUP = 4          # cores per batch group; also heads per core
TLOC = 512         # tokens owned per core (x_T input / final output shard)
TB = 2048          # tokens per batch (all local; x replicated per batch)
HG = 4             # heads per core
OSL = H // NC      # 256: per-core slice of Wp output dims (8-way for AG)
Q_MUL = 1.0 / math.sqrt(D)
RMAGIC = 12582912.0  # 1.5 * 2**23

_COMPILED = {}


def _build():
    import concourse.mybir as mybir
    import concourse.tile as tile
    from concourse import bacc
    from concourse import bass_isa

    f32 = mybir.dt.float32
    bf16 = mybir.dt.bfloat16
    AF = mybir.ActivationFunctionType
    OP = mybir.AluOpType
    AX = mybir.AxisListType

    nc = bacc.Bacc(None, target_bir_lowering=False, debug=False, num_devices=NC)

    x_T = nc.declare_dram_parameter("x_T", [H, TLOC], f32, isOutput=False)
    x_b_T = nc.declare_dram_parameter("x_b_T", [H, TB], f32, isOutput=False)
    wqkv = nc.declare_dram_parameter("wqkv", [3, H, 512], f32, isOutput=False)
    wp_sl = nc.declare_dram_parameter("wp_sl", [H, OSL], f32, isOutput=False)
    bqk = nc.declare_dram_parameter("bqk", [128, 8], f32, isOutput=False)
    bv_b = nc.declare_dram_parameter("bv_b", [128, 512], f32, isOutput=False)
    bp_t = nc.declare_dram_parameter("bp_t", [128, 16], f32, isOutput=False)
    bmask = nc.declare_dram_parameter("bmask", [1, 2], f32, isOutput=False)
    out_ext = nc.declare_dram_parameter("out", [H, TLOC], f32, isOutput=True)

    allg = [list(range(NC))]

    with tile.TileContext(nc) as tc, ExitStack() as top:
        dram = top.enter_context(tc.tile_pool(name="dram", bufs=1, space="DRAM"))
        wp_bounce = dram.tile([2, 128, 16, 128], bf16, name="wpb")
        wp_g = dram.tile([NC, 2, 128, 16, 128], bf16, addr_space="Shared", name="wpg")
        kdeq_sp = dram.tile([16, 128, TLOC], f32, name="kdsp")
        qdeq_sp = dram.tile([16, 128, TLOC], f32, name="qdsp")
        vdeq_sp = dram.tile([16, 128, TLOC], f32, name="vdsp")
        xint_d = dram.tile([GROUP, 128, 16, TLOC], bf16, name="xintd")
        ar1a_i = dram.tile([1, 1], f32)
        ar1a_o = dram.tile([1, 1], f32, addr_space="Shared")
        ar1b_i = dram.tile([1, 1], f32)
        ar1b_o = dram.tile([1, 1], f32, addr_space="Shared")
        ar1c_i = dram.tile([1, 1], f32)
        ar1c_o = dram.tile([1, 1], f32, addr_space="Shared")
        ar1d_i = dram.tile([1, 2], f32)
        ar1d_o = dram.tile([1, 2], f32, addr_space="Shared")
        ar2_i = dram.tile([1, 2], f32)
        ar2_o = dram.tile([1, 2], f32, addr_space="Shared")
        ar3a_i = dram.tile([1, 1], f32)
        ar3a_o = dram.tile([1, 1], f32, addr_space="Shared")
        ar3b_i = dram.tile([1, 1], f32)
        ar3b_o = dram.tile([1, 1], f32, addr_space="Shared")
        ar4_i = dram.tile([1, 1], f32)
        ar4_o = dram.tile([1, 1], f32, addr_space="Shared")
        a2a_in = dram.tile([NC, 128, HG, TLOC], bf16, name="a2ai")
        a2a_out = dram.tile([NC, 128, HG, TLOC], bf16, name="a2ao")

        const = top.enter_context(tc.tile_pool(name="const", bufs=1))
        sc = top.enter_context(tc.tile_pool(name="scal", bufs=1))
        sbuf = top.enter_context(tc.tile_pool(name="sbuf_main", bufs=1))
        qpool = top.enter_context(tc.tile_pool(name="qscratch", bufs=3))

        ones3 = const.tile([3, 128], bf16)
        nc.vector.memset(ones3[:], 1.0)
        bqk_sb = const.tile([128, 8], f32)
        nc.sync.dma_start(out=bqk_sb[:], in_=bqk[:, :])
        bv_sb = const.tile([128, 512], f32)
        nc.sync.dma_start(out=bv_sb[:], in_=bv_b[:, :])
        bp_sb = const.tile([128, 16], f32)
        nc.sync.dma_start(out=bp_sb[:], in_=bp_t[:, :])
        bm_sb = const.tile([1, 2], f32)
        nc.sync.dma_start(out=bm_sb[:], in_=bmask[:, :])

        scal = sc.tile([128, 96], f32, name="scal")
        _col = [0]

        def cols(n):
            c0 = _col[0]
            _col[0] += n
            assert _col[0] <= 96
            return scal[:, c0 : c0 + n]

        parts = sc.tile([128, 192], f32, name="parts")
        _pcol = [0]

        def pcols(n):
            c0 = _pcol[0]
            _pcol[0] += n
            assert _pcol[0] <= 192
            return parts[:, c0 : c0 + n]

        def p_reduce_max(part_col):
            red = cols(1)
            nc.gpsimd.partition_all_reduce(
                red, part_col, channels=128, reduce_op=bass_isa.ReduceOp.max
            )
            return red

        def bcast(src1n):
            b = cols(src1n.shape[-1])
            nc.gpsimd.partition_broadcast(b, src1n)
            return b

        def rnd(out_ap, in_ap):
            nc.vector.tensor_scalar(
                out_ap, in_ap, RMAGIC, RMAGIC, op0=OP.add, op1=OP.subtract
            )

        # long-lived singles (stack/LIFO order)
        q_int, q_int_free = tc.tile([128, HG, TB], bf16, name="q_int")
        k_int, k_int_free = tc.tile([128, HG, TB], bf16, name="k_int")
        v_int, v_int_free = tc.tile([128, 16, TLOC], bf16, name="v_int")

        wqkv_r = wqkv.rearrange("w (ht p) o -> w p ht o", p=128)
        wp_r = wp_sl.rearrange("(ht p) o -> p ht o", p=128)
        xb_r = x_b_T.rearrange("(ht p) t -> p ht t", p=128)

        with tc.tile_pool(name="wvint", bufs=1) as wv_pool:
            wv_i = wv_pool.tile([128, 16, 512], bf16, name="wv_i")
            xch_ctx = ExitStack()
            xch_pool = xch_ctx.enter_context(tc.tile_pool(name="xch", bufs=2))
            wqk_ctx = ExitStack()
            wqk_pool = wqk_ctx.enter_context(tc.tile_pool(name="wqkint", bufs=1))
            wq_i = wqk_pool.tile([128, 16, 512], bf16, name="wq_i")
            wk_i = wqk_pool.tile([128, 16, 512], bf16, name="wk_i")

            with tc.tile_pool(name="wf", bufs=4) as wf_pool:
                # ====== P0: x_T amax -> AR1a; W slices amax -> AR1b ======
                xT_r = x_T.rearrange("(ht p) t -> p ht t", p=128)
                xparts = pcols(16)
                for ht in range(16):
                    xf = wf_pool.tile([128, 512], f32, name="wf")
                    nc.sync.dma_start(out=xf[:], in_=xT_r[:, ht, :])
                    nc.vector.tensor_reduce(
                        xparts[:, ht : ht + 1], xf[:], AX.X, OP.max,
                        apply_absolute_value=True,
                    )
                xa = pcols(1)
                nc.vector.tensor_reduce(xa, xparts, AX.X, OP.max)
                xag = p_reduce_max(xa)
                nc.sync.dma_start(out=ar1a_i[:], in_=xag[0:1, :])
                nc.gpsimd.collective_compute(
                    "AllReduce", OP.max, replica_groups=allg,
                    ins=[ar1a_i[:].opt()], outs=[ar1a_o[:].opt()],
                )
                wap = pcols(16)

                def w_amax(w, dst_i, slot, eng=None):
                    eng = eng or nc.sync
                    wid = 512 if w < 3 else OSL
                    for ht in range(16):
                        wf = wf_pool.tile([128, 512], f32, name="wf")
                        src = wqkv_r[w, :, ht, :] if w < 3 else wp_r[:, ht, :]
                        eng.dma_start(out=wf[:, :wid], in_=src)
                        nc.vector.tensor_reduce(
                            wap[:, ht : ht + 1], wf[:, :wid], AX.X, OP.max,
                            apply_absolute_value=True,
                        )
                    wa = pcols(1)
                    nc.vector.tensor_reduce(wa, wap, AX.X, OP.max)
                    wag = p_reduce_max(wa)
                    nc.sync.dma_start(out=dst_i[:, slot : slot + 1], in_=wag[0:1, :])

                w_amax(0, ar1b_i, 0)
                nc.gpsimd.collective_compute(
                    "AllReduce", OP.max, replica_groups=allg,
                    ins=[ar1b_i[:].opt()], outs=[ar1b_o[:].opt()],
                )
                w_amax(1, ar1c_i, 0)
                nc.gpsimd.collective_compute(
                    "AllReduce", OP.max, replica_groups=allg,
                    ins=[ar1c_i[:].opt()], outs=[ar1c_o[:].opt()],
                )

                # ====== P1: scales; quantize wq/wk; x chunk quant + P2 below
                g5 = cols(5)[0:1, :]
                s5 = cols(5)[0:1, :]
                i5 = cols(5)[0:1, :]
                i5b = cols(5)
                sxw = cols(3)[0:1, :]
                sxwb = cols(3)

                def scales_from(sl, src_t, src_sl):
                    nc.sync.dma_start(out=g5[:, sl], in_=src_t[:, src_sl])
                    nc.vector.tensor_scalar(
                        s5[:, sl], g5[:, sl], 1.0 / 127.0, 1e-8,
                        op0=OP.mult, op1=OP.max,
                    )
                    nc.vector.reciprocal(i5[:, sl], s5[:, sl])
                    nc.gpsimd.partition_broadcast(i5b[:, sl], i5[:, sl])

                scales_from(slice(0, 1), ar1a_o, slice(0, 1))
                scales_from(slice(1, 2), ar1b_o, slice(0, 1))
                scales_from(slice(2, 3), ar1c_o, slice(0, 1))
                for w in range(2):
                    nc.vector.tensor_mul(
                        sxw[:, w : w + 1], s5[:, 0:1], s5[:, 1 + w : 2 + w]
                    )
                nc.gpsimd.partition_broadcast(sxwb[:, 0:2], sxw[:, 0:2])

                # x quant chunk 0 early (gates first projections)
                def x_quant_chunk(r):
                    xc = xch_pool.tile([128, 16, TLOC], bf16, name="xc")
                    for ht in range(16):
                        xf = wf_pool.tile([128, 512], f32, name="xf")
                        nc.sync.dma_start(
                            out=xf[:], in_=xb_r[:, ht, r * TLOC : (r + 1) * TLOC]
                        )
                        xm = qpool.tile([128, 512], f32, name="xm", tag="qs_f32")
                        nc.scalar.activation(
                            xm[:], xf[:], AF.Copy, scale=i5b[:, 0:1]
                        )
                        rnd(xc[:, ht, :], xm[:])
                    nc.scalar.dma_start(out=xint_d[r], in_=xc[:])
                    return xc

                xc0 = x_quant_chunk(0)

                def w_quant(w, dst):
                    for ht in range(16):
                        wf = wf_pool.tile([128, 512], f32, name="wf2")
                        nc.sync.dma_start(out=wf[:], in_=wqkv_r[w, :, ht, :])
                        wm = qpool.tile([128, 512], f32, name="wm", tag="qs_f32")
                        nc.scalar.activation(
                            wm[:], wf[:], AF.Copy, scale=i5b[:, 1 + w : 2 + w]
                        )
                        rnd(dst[:, ht, :], wm[:])

                w_quant(0, wq_i)
                w_quant(1, wk_i)

                # wv/wp amax -> AR1d (off the critical path, overlaps P2)
                w_amax(2, ar1d_i, 0, eng=nc.scalar)
                w_amax(3, ar1d_i, 1, eng=nc.scalar)
                nc.gpsimd.collective_compute(
                    "AllReduce", OP.max, replica_groups=allg,
                    ins=[ar1d_i[:].opt()], outs=[ar1d_o[:].opt()],
                )
                scales_from(slice(3, 5), ar1d_o, slice(0, 2))
                nc.vector.tensor_mul(sxw[:, 2:3], s5[:, 0:1], s5[:, 3:4])
                nc.gpsimd.partition_broadcast(sxwb[:, 2:3], sxw[:, 2:3])

                # ====== P2: q/k projections (d-major); q+k spilled to DRAM
                qa_parts = pcols(16)
                ka_parts = pcols(16)
                with tc.tile_pool(name="kev", bufs=4) as kev_pool, \
                     tc.tile_pool(name="qk_psum", bufs=6, space="PSUM") as qk_psum:
                    for r in range(GROUP):
                        xin = xc0 if r == 0 else x_quant_chunk(r)
                        for wi_, bcol, scol, aparts, spill in (
                            (wq_i, 0, 0, qa_parts, qdeq_sp),
                            (wk_i, 4, 1, ka_parts, kdeq_sp),
                        ):
                            for m in range(HG):
                                ps = qk_psum.tile([128, TLOC], f32, name="ps_qk")
                                for ht in range(16):
                                    nc.tensor.matmul(
                                        ps[:], wi_[:, ht, m * 128 : (m + 1) * 128],
                                        xin[:, ht, :],
                                        start=(ht == 0), stop=(ht == 15),
                                    )
                                kev = kev_pool.tile([128, TLOC], f32, name="kev")
                                nc.scalar.activation(
                                    kev[:], ps[:], AF.Identity,
                                    scale=sxwb[:, scol : scol + 1],
                                    bias=bqk_sb[:, bcol + m : bcol + m + 1],
                                )
                                nc.vector.tensor_reduce(
                                    aparts[:, r * 4 + m : r * 4 + m + 1],
                                    kev[:], AX.X, OP.max,
                                    apply_absolute_value=True,
                                )
                                nc.scalar.dma_start(
                                    out=spill[m * 4 + r], in_=kev[:]
                                )

                # ====== P3: AR2 (q,k amax); wv/wp quant in the AR2 window
                for i, prt in enumerate((qa_parts, ka_parts)):
                    acol = pcols(1)
                    nc.vector.tensor_reduce(acol, prt, AX.X, OP.max)
                    ag_ = p_reduce_max(acol)
                    nc.sync.dma_start(out=ar2_i[:, i : i + 1], in_=ag_[0:1, :])
                nc.gpsimd.collective_compute(
                    "AllReduce", OP.max, replica_groups=allg,
                    ins=[ar2_i[:].opt()], outs=[ar2_o[:].opt()],
                )
                w_quant(2, wv_i)
            wqk_ctx.close()

            # ====== v projection fills the AR2 window (needs no AR2),
            # interleaved with per-head q/k quant so pass-1 h0 starts early
            va_parts = pcols(16)
            with tc.tile_pool(name="rld", bufs=16) as rld_pool, \
                 tc.tile_pool(name="rldq", bufs=16) as rldq_pool, \
                 tc.tile_pool(name="v_psum", bufs=3, space="PSUM") as v_psum, \
                 tc.tile_pool(name="vev", bufs=4) as vev_pool:
                klds = []
                for i in range(16):
                    kld = rld_pool.tile([128, TLOC], f32, name="rld")
                    nc.scalar.dma_start(out=kld[:], in_=kdeq_sp[i])
                    klds.append(kld)
                qlds = []
                for i in range(16):
                    qld = rldq_pool.tile([128, TLOC], f32, name="rldq")
                    nc.scalar.dma_start(out=qld[:], in_=qdeq_sp[i])
                    qlds.append(qld)

                def v_chunk(h):
                    xv = xch_pool.tile([128, 16, TLOC], bf16, name="xc")
                    nc.sync.dma_start(out=xv[:], in_=xint_d[h])
                    for ts in range(4):
                        tsl = h * 4 + ts
                        ps = v_psum.tile([128, 512], f32, name="ps_v")
                        for ht in range(16):
                            nc.tensor.matmul(
                                ps[:], xv[:, ht, ts * 128 : (ts + 1) * 128],
                                wv_i[:, ht, :],
                                start=(ht == 0), stop=(ht == 15),
                            )
                        vtmp = qpool.tile([128, 512], f32, name="vtmp", tag="qs_f32")
                        nc.scalar.activation(
                            vtmp[:], ps[:], AF.Copy, scale=sxwb[:, 2:3]
                        )
                        vev = vev_pool.tile([128, 512], f32, name="vev")
                        nc.vector.tensor_add(vev[:], vtmp[:], bv_sb[:])
                        nc.vector.tensor_reduce(
                            va_parts[:, tsl : tsl + 1], vev[:], AX.X, OP.max,
                            apply_absolute_value=True,
                        )
                        nc.scalar.dma_start(out=vdeq_sp[tsl], in_=vev[:])

                def qk_quant(hm):
                    for r in range(GROUP):
                        sl = slice(r * TLOC, (r + 1) * TLOC)
                        m2 = qpool.tile([128, TLOC], f32, name="km", tag="qs_f32")
                        nc.scalar.activation(
                            m2[:], klds[hm * 4 + r][:], AF.Copy, scale=qf2b[:, 1:2]
                        )
                        rnd(k_int[:, hm, sl], m2[:])
                    for r in range(GROUP):
                        sl = slice(r * TLOC, (r + 1) * TLOC)
                        m = qpool.tile([128, TLOC], f32, name="qm", tag="qs_f32")
                        nc.scalar.activation(
                            m[:], qlds[hm * 4 + r][:], AF.Copy, scale=qf2b[:, 0:1]
                        )
                        rnd(q_int[:, hm, sl], m[:])

                v_chunk(0)
                v_chunk(1)

                g2 = cols(2)[0:1, :]
                nc.sync.dma_start(out=g2, in_=ar2_o[:])
                s_q = cols(1)[0:1, :]
                nc.vector.tensor_scalar(
                    s_q, g2[:, 0:1], Q_MUL / 127.0, 1e-8,
                    op0=OP.mult, op1=OP.max,
                )
                qf = cols(1)[0:1, :]
                nc.vector.reciprocal(qf, s_q)
                nc.vector.tensor_scalar_mul(qf, qf, Q_MUL)
                s_k = cols(1)[0:1, :]
                nc.vector.tensor_scalar(
                    s_k, g2[:, 1:2], 1.0 / 127.0, 1e-8, op0=OP.mult, op1=OP.max
                )
                i_k = cols(1)[0:1, :]
                nc.vector.reciprocal(i_k, s_k)
                s_qk = cols(1)[0:1, :]
                nc.vector.tensor_mul(s_qk, s_q, s_k)
                qf2 = cols(2)[0:1, :]
                nc.vector.tensor_copy(qf2[:, 0:1], qf)
                nc.vector.tensor_copy(qf2[:, 1:2], i_k)
                qf2b = bcast(qf2)
                s_qk_b = bcast(s_qk)
                neg_inv_sqk = cols(1)[0:1, :]
                nc.vector.reciprocal(neg_inv_sqk, s_qk)
                nc.vector.tensor_scalar_mul(neg_inv_sqk, neg_inv_sqk, -1.0)
                nis_b = bcast(neg_inv_sqk)

                qk_quant(0)
                v_chunk(2)
                qk_quant(1)
                v_chunk(3)
                qk_quant(2)
                qk_quant(3)
            xch_ctx.close()
            va = pcols(1)
            nc.vector.tensor_reduce(va, va_parts, AX.X, OP.max)
            vag = p_reduce_max(va)
            nc.sync.dma_start(out=ar3b_i[:], in_=vag[0:1, :])
            nc.gpsimd.collective_compute(
                "AllReduce", OP.max, replica_groups=allg,
                ins=[ar3b_i[:].opt()], outs=[ar3b_o[:].opt()],
            )


            # ====== P4: pass 1 (stats)
            stats = sbuf.tile([128, 512], f32, name="stats")
            S_all = stats[:, 0:64]
            M_all = stats[:, 64:128]
            with tc.tile_pool(name="p4_psum", bufs=2, space="PSUM") as p4_psum, \
                 tc.tile_pool(name="epool", bufs=2) as e_pool:
                for h in range(HG):
                    for qt in range(16):
                        ps = p4_psum.tile([128, TB], f32, name="ps_a")
                        for jc in range(4):
                            nc.tensor.matmul(
                                ps[:, jc * 512 : (jc + 1) * 512],
                                q_int[:, h, qt * 128 : (qt + 1) * 128],
                                k_int[:, h, jc * 512 : (jc + 1) * 512],
                                start=True, stop=True,
                            )
                        col = h * 16 + qt
                        E = e_pool.tile([128, TB], f32, name="E")
                        nc.scalar.activation(
                            E[:], ps[:], AF.Exp, scale=s_qk_b[:, 0:1],
                            accum_out=S_all[:, col : col + 1],
                        )
                        nc.vector.tensor_scalar(
                            E[:], E[:], 1.0, None, op0=OP.mult,
                            op1=OP.max, accum_out=M_all[:, col : col + 1],
                        )

            # ====== P5a: stats -> AR3a (R)
            Sinv = stats[:, 128:192]
            nc.vector.reciprocal(Sinv, S_all)
            R = stats[:, 192:256]
            nc.vector.tensor_mul(R, M_all, Sinv)
            ra = pcols(1)
            nc.vector.tensor_reduce(ra, R, AX.X, OP.max)
            rag = p_reduce_max(ra)
            nc.sync.dma_start(out=ar3a_i[:], in_=rag[0:1, :])
            nc.gpsimd.collective_compute(
                "AllReduce", OP.max, replica_groups=allg,
                ins=[ar3a_i[:].opt()], outs=[ar3a_o[:].opt()],
            )

            # wp quant + Wp AllGather: off the critical path, after q/k quant
            with tc.tile_pool(name="wvp", bufs=4) as wvp_pool:
                wpb_r = wp_bounce.rearrange("half p hc o -> p half hc o")
                for ht in range(16):
                    wf = wvp_pool.tile([128, 512], f32, name="wvp")
                    nc.sync.dma_start(out=wf[:, :OSL], in_=wp_r[:, ht, :])
                    wm = qpool.tile([128, 512], f32, name="wpm", tag="qs_f32")
                    nc.scalar.activation(
                        wm[:, :OSL], wf[:, :OSL], AF.Copy, scale=i5b[:, 4:5]
                    )
                    wi = qpool.tile([128, OSL], bf16, name="wpi", tag="qs_bf16")
                    rnd(wi[:], wm[:, :OSL])
                    nc.sync.dma_start(
                        out=wpb_r[:, :, ht, :],
                        in_=wi[:].rearrange("p (half o) -> p half o", o=128),
                    )
                nc.gpsimd.collective_compute(
                    "AllGather", OP.bypass, replica_groups=allg,
                    ins=[wp_bounce[:].opt()], outs=[wp_g[:].opt()],
                )


        # c' = -ln(S_i)/s_qk, prepared during AR3a.  Computed on [128, 64],
        # xbar-transposed to [64, 128], hi/mid/lo split there, then flattened
        # into the resident cmv_all [3, HG*TB] (row r=(h*16+qt), col p).
        cl = stats[:, 320:384]
        nc.scalar.activation(cl, S_all, AF.Ln)
        cpr = stats[:, 384:448]
        nc.vector.tensor_scalar(cpr, cl, nis_b[:, 0:1], None, op0=OP.mult)
        chib = sbuf.tile([128, 128], bf16, name="chib")
        cmib = sbuf.tile([128, 128], bf16, name="cmib")
        clob = sbuf.tile([128, 128], bf16, name="clob")
        nc.vector.memset(chib[:, 64:], 0.0)
        nc.vector.memset(cmib[:, 64:], 0.0)
        nc.vector.memset(clob[:, 64:], 0.0)
        chif = stats[:, 448:512]
        res1 = stats[:, 128:192]
        res2 = stats[:, 192:256]
        nc.vector.tensor_copy(chib[:, :64], cpr)
        nc.vector.tensor_copy(chif, chib[:, :64])
        nc.vector.tensor_sub(res1, cpr, chif)
        nc.vector.tensor_copy(cmib[:, :64], res1)
        nc.vector.tensor_copy(chif, cmib[:, :64])
        nc.vector.tensor_sub(res2, res1, chif)
        nc.vector.tensor_copy(clob[:, :64], res2)
        # xbar-transpose each bf16 split to [64(h qt), 128(p)], then flatten
        # rows into the resident cmv_all [3, HG*TB] (index h*2048+qt*128+p)
        cT0 = sbuf.tile([128, 128], bf16, name="cT0")
        cT1 = sbuf.tile([128, 128], bf16, name="cT1")
        cT2 = sbuf.tile([128, 128], bf16, name="cT2")
        cmv_all = sbuf.tile([3, HG * TB], bf16, name="cmv_all")
        for ci, t, ct in ((0, chib, cT0), (1, cmib, cT1), (2, clob, cT2)):
            nc.sync.dma_start_transpose(out=ct[:], in_=t[:])
            nc.sync.dma_start(
                out=cmv_all[ci : ci + 1, :].rearrange("o (r c) -> o r c", c=128),
                in_=ct[0:64, :],
            )

        gA = cols(1)[0:1, :]
        nc.sync.dma_start(out=gA, in_=ar3a_o[:])
        s_attn = cols(1)[0:1, :]
        nc.vector.tensor_scalar(
            s_attn, gA, 1.0 / 127.0, 1e-8, op0=OP.mult, op1=OP.max
        )
        lnsa = cols(1)[0:1, :]
        nc.scalar.activation(lnsa, s_attn, AF.Ln)
        nc.vector.tensor_scalar_mul(lnsa, lnsa, -1.0)
        eb_b = bcast(lnsa)
        gV = cols(1)[0:1, :]
        nc.sync.dma_start(out=gV, in_=ar3b_o[:])
        s_v = cols(1)[0:1, :]
        nc.vector.tensor_scalar(s_v, gV, 1.0 / 127.0, 1e-8, op0=OP.mult, op1=OP.max)
        i_v = cols(1)[0:1, :]
        nc.vector.reciprocal(i_v, s_v)
        i_v_b = bcast(i_v)
        s_av = cols(1)[0:1, :]
        nc.vector.tensor_mul(s_av, s_attn, s_v)
        s_av_b = bcast(s_av)

        # ====== P6: pass 2 + S@V (v quantized from spill after AR3b)
        out_T, out_T_free = tc.tile([128, HG, TB], f32, name="out_T")
        oa_parts = pcols(16)
        vq_done = [False]
        with tc.tile_pool(name="p6_psum", bufs=3, space="PSUM") as p6_psum, \
             tc.tile_pool(name="sv_psum", bufs=2, space="PSUM") as sv_psum, \
             tc.tile_pool(name="pint", bufs=2) as pint_pool, \
             tc.tile_pool(name="ps_scr", bufs=3) as ps_scr, \
             tc.tile_pool(name="vld", bufs=3) as vld_pool:
            for h in range(HG):
                cmv = cmv_all[:, h * TB : (h + 1) * TB]
                for qcp in range(2):
                    p_int = pint_pool.tile([128, 16, 1024], bf16, name="p_int")
                    for jt in range(16):
                        ps2 = p6_psum.tile([128, 1024], f32, name="ps2")
                        for half in range(2):
                            q0 = qcp * 1024 + half * 512
                            nc.tensor.matmul(
                                ps2[:, half * 512 : (half + 1) * 512],
                                k_int[:, h, jt * 128 : (jt + 1) * 128],
                                q_int[:, h, q0 : q0 + 512],
                                start=True, stop=False,
                            )
                        for half in range(2):
                            q0 = qcp * 1024 + half * 512
                            nc.tensor.matmul(
                                ps2[:, half * 512 : (half + 1) * 512],
                                ones3[:], cmv[:, q0 : q0 + 512],
                                start=False, stop=True, skip_group_check=True,
                            )
                        PS = ps_scr.tile([128, 1024], f32, name="PS")
                        nc.scalar.activation(
                            PS[:], ps2[:], AF.Exp,
                            scale=s_qk_b[:, 0:1], bias=eb_b[:, 0:1],
                        )
                        rnd(p_int[:, jt, :], PS[:])
                    if not vq_done[0]:
                        # quantize v now (AR3b has landed); needed by first S@V
                        for tsl in range(16):
                            vld = vld_pool.tile([128, 512], f32, name="vld")
                            nc.scalar.dma_start(out=vld[:], in_=vdeq_sp[tsl])
                            m = qpool.tile([128, 512], f32, name="vm", tag="qs_f32")
                            nc.scalar.activation(
                                m[:], vld[:], AF.Copy, scale=i_v_b[:, 0:1]
                            )
                            rnd(v_int[:, tsl, :], m[:])
                        vq_done[0] = True
                    for qch in range(2):
                        q0 = qcp * 1024 + qch * 512
                        ps3 = sv_psum.tile([128, 512], f32, name="ps3")
                        for jt in range(16):
                            nc.tensor.matmul(
                                ps3[:], v_int[:, jt, h * 128 : (h + 1) * 128],
                                p_int[:, jt, qch * 512 : (qch + 1) * 512],
                                start=(jt == 0), stop=(jt == 15),
                            )
                        nc.scalar.activation(
                            out_T[:, h, q0 : q0 + 512], ps3[:], AF.Copy,
                            scale=s_av_b[:, 0:1],
                        )
                        oi = h * 4 + qcp * 2 + qch
                        nc.vector.tensor_reduce(
                            oa_parts[:, oi : oi + 1],
                            out_T[:, h, q0 : q0 + 512], AX.X, OP.max,
                            apply_absolute_value=True,
                        )

        # ====== P7: out amax -> AR4 -> masked quantize -> A2A
        oc_ = pcols(1)
        nc.vector.tensor_reduce(oc_, oa_parts, AX.X, OP.max)
        ocg = p_reduce_max(oc_)
        nc.sync.dma_start(out=ar4_i[:], in_=ocg[0:1, :])
        nc.gpsimd.collective_compute(
            "AllReduce", OP.max, replica_groups=allg,
            ins=[ar4_i[:].opt()], outs=[ar4_o[:].opt()],
        )
        gO = cols(1)[0:1, :]
        nc.sync.dma_start(out=gO, in_=ar4_o[:])
        s_out = cols(1)[0:1, :]
        nc.vector.tensor_scalar(s_out, gO, 1.0 / 127.0, 1e-8, op0=OP.mult, op1=OP.max)
        i_out = cols(1)[0:1, :]
        nc.vector.reciprocal(i_out, s_out)
        s_op = cols(1)[0:1, :]
        nc.vector.tensor_mul(s_op, s_out, s5[:, 4:5])
        s_op_b = bcast(s_op)
        iobm = cols(2)[0:1, :]
        nc.vector.tensor_mul(iobm[:, 0:1], i_out, bm_sb[:, 0:1])
        nc.vector.tensor_mul(iobm[:, 1:2], i_out, bm_sb[:, 1:2])
        iobm_b = bcast(iobm)

        # quantize out into the 8 A2A shards: shard j = token chunk j%4,
        # scaled by bmask[j//4] (exact zeros for the other batch's shards)
        with tc.tile_pool(name="om2", bufs=2) as om2_pool, \
             tc.tile_pool(name="a2ast", bufs=3) as a2ast_pool:
            for j in range(NC):
                ch = j % 4
                csl = slice(ch * TLOC, (ch + 1) * TLOC)
                m = om2_pool.tile([128, HG, TLOC], f32, name="om")
                nc.scalar.activation(
                    m[:], out_T[:, :, csl], AF.Copy,
                    scale=iobm_b[:, j // 4 : j // 4 + 1],
                )
                st = a2ast_pool.tile([128, HG, TLOC], bf16, name="a2st")
                rnd(st[:], m[:])
                nc.scalar.dma_start(out=a2a_in[j], in_=st[:])
        nc.gpsimd.collective_compute(
            "AllToAll", OP.bypass, replica_groups=allg,
            ins=[a2a_in[:].opt()], outs=[a2a_out[:].opt()],
        )
        out_T_free()

        # ====== P8: output projection
        out_r = out_ext.rearrange("(ot p) t -> p ot t", p=128)
        a2a_lo = a2a_out[0:GROUP].rearrange("s p h t -> p s h t")
        a2a_hi = a2a_out[GROUP:NC].rearrange("s p h t -> p s h t")
        with tc.tile_pool(name="p8_psum", bufs=4, space="PSUM") as p8_psum, \
             tc.tile_pool(name="wcol8", bufs=3) as wcol_pool, \
             tc.tile_pool(name="fin", bufs=3) as fin_pool, \
             tc.tile_pool(name="oin", bufs=1) as oin_pool:
            oin_a = oin_pool.tile([128, GROUP, HG, TLOC], bf16, name="oin_a")
            nc.sync.dma_start(out=oin_a[:], in_=a2a_lo)
            oin_b = oin_pool.tile([128, GROUP, HG, TLOC], bf16, name="oin_b")
            nc.sync.dma_start(out=oin_b[:], in_=a2a_hi)
            oin4 = oin_pool.tile([128, GROUP, HG, TLOC], bf16, name="oin")
            nc.vector.tensor_add(oin4[:], oin_a[:], oin_b[:])
            oin = oin4.rearrange("p s h t -> p (s h) t")
            for ot in range(16):
                wcol = wcol_pool.tile([128, 16, 128], bf16, name="wcol")
                nc.sync.dma_start(out=wcol[:], in_=wp_g[ot // 2, ot % 2])
                ps = p8_psum.tile([128, TLOC], f32, name="ps_p")
                for hc in range(16):
                    nc.tensor.matmul(
                        ps[:], wcol[:, hc, :], oin[:, hc, :],
                        start=(hc == 0), stop=(hc == 15),
                    )
                fin = fin_pool.tile([128, TLOC], f32, name="fin")
                nc.scalar.activation(
                    fin[:], ps[:], AF.Identity,
                    scale=s_op_b[:, 0:1], bias=bp_sb[:, ot : ot + 1],
                )
                nc.sync.dma_start(out=out_r[:, ot, :], in_=fin[:])

        v_int_free()
        k_int_free()
        q_int_free()

    nc.compile()
    return nc


def _get_compiled():
    if "nc" not in _COMPILED:
        _COMPILED["nc"] = _build()
    return _COMPILED["nc"]


def make_in_maps(hidden_states, Wq, bq, Wk, bk, Wv, bv, Wp, bp):
    hs = np.asarray(hidden_states, dtype=np.float32)
    wT = [
        np.ascontiguousarray(np.asarray(W, np.float32).T)
        for W in (Wq, Wk, Wv, Wp)
    ]
    bq_ = np.asarray(bq, np.float32)
    bk_ = np.asarray(bk, np.float32)
    bv_ = np.asarray(bv, np.float32)
    bp_t = np.ascontiguousarray(np.asarray(bp, np.float32).reshape(16, 128).T)
    xbT = [np.ascontiguousarray(hs[b].T) for b in range(B)]
    in_maps = []
    for c in range(NC):
        b = c // GROUP
        g = c % GROUP
        x_Tc = np.ascontiguousarray(hs[b, g * TLOC : (g + 1) * TLOC, :].T)
        dsl = slice(512 * g, 512 * (g + 1))
        wqkv_c = np.ascontiguousarray(
            np.stack([wT[0][:, dsl], wT[1][:, dsl], wT[2][:, dsl]], axis=0)
        )
        wp_c = np.ascontiguousarray(wT[3][:, c * OSL : (c + 1) * OSL])
        bqk_c = np.ascontiguousarray(
            np.concatenate(
                [bq_[dsl].reshape(4, 128).T, bk_[dsl].reshape(4, 128).T], axis=1
            )
        )
        bv_c = np.ascontiguousarray(
            np.broadcast_to(bv_[dsl][None, :], (128, 512))
        )
        bm = np.zeros((1, 2), np.float32)
        bm[0, b] = 1.0
        in_maps.append(
            {"x_T": x_Tc, "x_b_T": xbT[b], "wqkv": wqkv_c, "wp_sl": wp_c,
             "bqk": bqk_c, "bv_b": bv_c, "bp_t": bp_t, "bmask": bm}
        )
    return in_maps


def kernel(hidden_states, Wq, bq, Wk, bk, Wv, bv, Wp, bp):
    from concourse.bass_utils import run_bass_kernel_spmd

    trace = bool(int(os.environ.get("KERNEL_TRACE", "0")))
    nc = _get_compiled()
    in_maps = make_in_maps(hidden_states, Wq, bq, Wk, bk, Wv, bv, Wp, bp)
    res = run_bass_kernel_spmd(nc, in_maps, core_ids=list(range(NC)), trace=trace)
    kernel.last_exec_time_ns = res.exec_time_ns
    kernel.last_results = res.results

    out = np.empty((B, S, H), dtype=np.float32)
    for c in range(NC):
        b = c // GROUP
        t0 = (c % GROUP) * TLOC
        out[b, t0 : t0 + TLOC, :] = res.results[c]["out"].T
    return out


kernel.last_exec_time_ns = None
kernel.last_results = None


# revision 63
# speedup vs baseline: 1.0263x; 1.0263x over previous
"""Distributed TRN2 Bass kernel for a dynamic-int8-quantized transformer encoder
layer — head-sharded (tensor-parallel) variant, v3.

Sharding: core c owns batch b=c//4 and heads 4g..4g+3 (g=c%4), i.e. H-dims
[512g, 512g+512).  The full batch x is replicated to every core (input
layout), so each core quantizes its batch's x locally — no x AllGather.  Each
core quantizes its own Wq/Wk/Wv column slice (no weight AllGather); attention
is fully head-local.  Before the output projection a masked 8-rank AllToAll
(cross-batch shards scaled to exact zeros via a one-hot bmask input) returns
to token sharding; the receiver merges mirror shard pairs with one bf16 add.
Wp is quantized in 8 shards and AllGathered early.

Numerics identical to the validated baseline:
  - int8 fake-quant matmuls as integer-valued bf16 matmuls (fp32 PSUM).
  - softmax without max-subtraction; two-pass with global amax(probs) =
    max_i M_i/S_i; pass 2 recomputes QK^T and adds c'_i = -ln(S_i)/s_qk
    (bf16 hi+mid+lo split via a k=3 matmul), bias B = -ln(s_attn) in the exp.
  - round-to-nearest-even: x + 1.5*2^23 - 1.5*2^23 in fp32.
"""

import math
import os
import sys
from contextlib import ExitStack

import numpy as np

sys.path.insert(0, "/opt/trn_rl_repo")

B = 2
S = 2048
H = 2048
NH = 16
D = 128
NC = 8
GROUP = 4          # cores per batch group; also heads per core
TLOC = 512         # tokens owned per core (x_T input / final output shard)
TB = 2048          # tokens per batch (all local; x replicated per batch)
HG = 4             # heads per core
OSL = H // NC      # 256: per-core slice of Wp output dims (8-way for AG)
Q_MUL = 1.0 / math.sqrt(D)
RMAGIC = 12582912.0  # 1.5 * 2**23

_COMPILED = {}


def _build():
    import concourse.mybir as mybir
    import concourse.tile as tile
    from concourse import bacc
    from concourse import bass_isa

    f32 = mybir.dt.float32
    bf16 = mybir.dt.bfloat16
    AF = mybir.ActivationFunctionType
    OP = mybir.AluOpType
    AX = mybir.AxisListType

    nc = bacc.Bacc(None, target_bir_lowering=False, debug=False, num_devices=NC)

    x_T = nc.declare_dram_parameter("x_T", [H, TLOC], f32, isOutput=False)
    x_b_T = nc.declare_dram_parameter("x_b_T", [H, TB], f32, isOutput=False)
    wqkv = nc.declare_dram_parameter("wqkv", [3, H, 512], f32, isOutput=False)
    wp_sl = nc.declare_dram_parameter("wp_sl", [H, OSL], f32, isOutput=False)
    bqk = nc.declare_dram_parameter("bqk", [128, 8], f32, isOutput=False)
    bv_b = nc.declare_dram_parameter("bv_b", [128, 512], f32, isOutput=False)
    bp_t = nc.declare_dram_parameter("bp_t", [128, 16], f32, isOutput=False)
    bmask = nc.declare_dram_parameter("bmask", [1, 2], f32, isOutput=False)
    out_ext = nc.declare_dram_parameter("out", [H, TLOC], f32, isOutput=True)

    allg = [list(range(NC))]

    with tile.TileContext(nc) as tc, ExitStack() as top:
        dram = top.enter_context(tc.tile_pool(name="dram", bufs=1, space="DRAM"))
        wp_bounce = dram.tile([2, 128, 16, 128], bf16, name="wpb")
        wp_g = dram.tile([NC, 2, 128, 16, 128], bf16, addr_space="Shared", name="wpg")
        kdeq_sp = dram.tile([16, 128, TLOC], f32, name="kdsp")
        qdeq_sp = dram.tile([16, 128, TLOC], f32, name="qdsp")
        vdeq_sp = dram.tile([16, 128, TLOC], f32, name="vdsp")
        xint_d = dram.tile([GROUP, 128, 16, TLOC], bf16, name="xintd")
        ar1a_i = dram.tile([1, 1], f32)
        ar1a_o = dram.tile([1, 1], f32, addr_space="Shared")
        ar1b_i = dram.tile([1, 1], f32)
        ar1b_o = dram.tile([1, 1], f32, addr_space="Shared")
        ar1c_i = dram.tile([1, 1], f32)
        ar1c_o = dram.tile([1, 1], f32, addr_space="Shared")
        ar1d_i = dram.tile([1, 2], f32)
        ar1d_o = dram.tile([1, 2], f32, addr_space="Shared")
        ar2_i = dram.tile([1, 2], f32)
        ar2_o = dram.tile([1, 2], f32, addr_space="Shared")
        ar3a_i = dram.tile([1, 1], f32)
        ar3a_o = dram.tile([1, 1], f32, addr_space="Shared")
        ar3b_i = dram.tile([1, 1], f32)
        ar3b_o = dram.tile([1, 1], f32, addr_space="Shared")
        ar4_i = dram.tile([1, 1], f32)
        ar4_o = dram.tile([1, 1], f32, addr_space="Shared")
        a2a_in = dram.tile([NC, 128, HG, TLOC], bf16, name="a2ai")
        a2a_out = dram.tile([NC, 128, HG, TLOC], bf16, name="a2ao")

        const = top.enter_context(tc.tile_pool(name="const", bufs=1))
        sc = top.enter_context(tc.tile_pool(name="scal", bufs=1))
        sbuf = top.enter_context(tc.tile_pool(name="sbuf_main", bufs=1))
        qpool = top.enter_context(tc.tile_pool(name="qscratch", bufs=3))

        ones3 = const.tile([3, 128], bf16)
        nc.vector.memset(ones3[:], 1.0)
        bqk_sb = const.tile([128, 8], f32)
        nc.sync.dma_start(out=bqk_sb[:], in_=bqk[:, :])
        bv_sb = const.tile([128, 512], f32)
        nc.sync.dma_start(out=bv_sb[:], in_=bv_b[:, :])
        bp_sb = const.tile([128, 16], f32)
        nc.sync.dma_start(out=bp_sb[:], in_=bp_t[:, :])
        bm_sb = const.tile([1, 2], f32)
        nc.sync.dma_start(out=bm_sb[:], in_=bmask[:, :])

        scal = sc.tile([128, 96], f32, name="scal")
        _col = [0]

        def cols(n):
            c0 = _col[0]
            _col[0] += n
            assert _col[0] <= 96
            return scal[:, c0 : c0 + n]

        parts = sc.tile([128, 192], f32, name="parts")
        _pcol = [0]

        def pcols(n):
            c0 = _pcol[0]
            _pcol[0] += n
            assert _pcol[0] <= 192
            return parts[:, c0 : c0 + n]

        def p_reduce_max(part_col):
            red = cols(1)
            nc.gpsimd.partition_all_reduce(
                red, part_col, channels=128, reduce_op=bass_isa.ReduceOp.max
            )
            return red

        def bcast(src1n):
            b = cols(src1n.shape[-1])
            nc.gpsimd.partition_broadcast(b, src1n)
            return b

        def rnd(out_ap, in_ap):
            nc.vector.tensor_scalar(
                out_ap, in_ap, RMAGIC, RMAGIC, op0=OP.add, op1=OP.subtract
            )

        # long-lived singles (stack/LIFO order)
        q_int, q_int_free = tc.tile([128, HG, TB], bf16, name="q_int")
        k_int, k_int_free = tc.tile([128, HG, TB], bf16, name="k_int")
        v_int, v_int_free = tc.tile([128, 16, TLOC], bf16, name="v_int")

        wqkv_r = wqkv.rearrange("w (ht p) o -> w p ht o", p=128)
        wp_r = wp_sl.rearrange("(ht p) o -> p ht o", p=128)
        xb_r = x_b_T.rearrange("(ht p) t -> p ht t", p=128)

        with tc.tile_pool(name="wvint", bufs=1) as wv_pool:
            wv_i = wv_pool.tile([128, 16, 512], bf16, name="wv_i")
            xch_ctx = ExitStack()
            xch_pool = xch_ctx.enter_context(tc.tile_pool(name="xch", bufs=2))
            wqk_ctx = ExitStack()
            wqk_pool = wqk_ctx.enter_context(tc.tile_pool(name="wqkint", bufs=1))
            wq_i = wqk_pool.tile([128, 16, 512], bf16, name="wq_i")
            wk_i = wqk_pool.tile([128, 16, 512], bf16, name="wk_i")

            with tc.tile_pool(name="wf", bufs=4) as wf_pool:
                # ====== P0: x_T amax -> AR1a; W slices amax -> AR1b ======
                xT_r = x_T.rearrange("(ht p) t -> p ht t", p=128)
                xparts = pcols(16)
                for ht in range(16):
                    xf = wf_pool.tile([128, 512], f32, name="wf")
                    nc.sync.dma_start(out=xf[:], in_=xT_r[:, ht, :])
                    nc.vector.tensor_reduce(
                        xparts[:, ht : ht + 1], xf[:], AX.X, OP.max,
                        apply_absolute_value=True,
                    )
                xa = pcols(1)
                nc.vector.tensor_reduce(xa, xparts, AX.X, OP.max)
                xag = p_reduce_max(xa)
                nc.sync.dma_start(out=ar1a_i[:], in_=xag[0:1, :])
                nc.gpsimd.collective_compute(
                    "AllReduce", OP.max, replica_groups=allg,
                    ins=[ar1a_i[:].opt()], outs=[ar1a_o[:].opt()],
                )
                wap = pcols(16)

                def w_amax(w, dst_i, slot, eng=None):
                    eng = eng or nc.sync
                    wid = 512 if w < 3 else OSL
                    for ht in range(16):
                        wf = wf_pool.tile([128, 512], f32, name="wf")
                        src = wqkv_r[w, :, ht, :] if w < 3 else wp_r[:, ht, :]
                        eng.dma_start(out=wf[:, :wid], in_=src)
                        nc.vector.tensor_reduce(
                            wap[:, ht : ht + 1], wf[:, :wid], AX.X, OP.max,
                            apply_absolute_value=True,
                        )
                    wa = pcols(1)
                    nc.vector.tensor_reduce(wa, wap, AX.X, OP.max)
                    wag = p_reduce_max(wa)
                    nc.sync.dma_start(out=dst_i[:, slot : slot + 1], in_=wag[0:1, :])

                w_amax(0, ar1b_i, 0)
                nc.gpsimd.collective_compute(
                    "AllReduce", OP.max, replica_groups=allg,
                    ins=[ar1b_i[:].opt()], outs=[ar1b_o[:].opt()],
                )
                w_amax(1, ar1c_i, 0)
                nc.gpsimd.collective_compute(
                    "AllReduce", OP.max, replica_groups=allg,
                    ins=[ar1c_i[:].opt()], outs=[ar1c_o[:].opt()],
                )

                # ====== P1: scales; quantize wq/wk; x chunk quant + P2 below
                g5 = cols(5)[0:1, :]
                s5 = cols(5)[0:1, :]
                i5 = cols(5)[0:1, :]
                i5b = cols(5)
                sxw = cols(3)[0:1, :]
                sxwb = cols(3)

                def scales_from(sl, src_t, src_sl):
                    nc.sync.dma_start(out=g5[:, sl], in_=src_t[:, src_sl])
                    nc.vector.tensor_scalar(
                        s5[:, sl], g5[:, sl], 1.0 / 127.0, 1e-8,
                        op0=OP.mult, op1=OP.max,
                    )
                    nc.vector.reciprocal(i5[:, sl], s5[:, sl])
                    nc.gpsimd.partition_broadcast(i5b[:, sl], i5[:, sl])

                scales_from(slice(0, 1), ar1a_o, slice(0, 1))
                scales_from(slice(1, 2), ar1b_o, slice(0, 1))
                scales_from(slice(2, 3), ar1c_o, slice(0, 1))
                for w in range(2):
                    nc.vector.tensor_mul(
                        sxw[:, w : w + 1], s5[:, 0:1], s5[:, 1 + w : 2 + w]
                    )
                nc.gpsimd.partition_broadcast(sxwb[:, 0:2], sxw[:, 0:2])

                # x quant chunk 0 early (gates first projections)
                def x_quant_chunk(r):
                    xc = xch_pool.tile([128, 16, TLOC], bf16, name="xc")
                    for ht in range(16):
                        xf = wf_pool.tile([128, 512], f32, name="xf")
                        nc.sync.dma_start(
                            out=xf[:], in_=xb_r[:, ht, r * TLOC : (r + 1) * TLOC]
                        )
                        xm = qpool.tile([128, 512], f32, name="xm", tag="qs_f32")
                        nc.scalar.activation(
                            xm[:], xf[:], AF.Copy, scale=i5b[:, 0:1]
                        )
                        rnd(xc[:, ht, :], xm[:])
                    nc.scalar.dma_start(out=xint_d[r], in_=xc[:])
                    return xc

                xc0 = x_quant_chunk(0)

                def w_quant(w, dst):
                    for ht in range(16):
                        wf = wf_pool.tile([128, 512], f32, name="wf2")
                        nc.sync.dma_start(out=wf[:], in_=wqkv_r[w, :, ht, :])
                        wm = qpool.tile([128, 512], f32, name="wm", tag="qs_f32")
                        nc.scalar.activation(
                            wm[:], wf[:], AF.Copy, scale=i5b[:, 1 + w : 2 + w]
                        )
                        rnd(dst[:, ht, :], wm[:])

                w_quant(0, wq_i)
                w_quant(1, wk_i)

                # wv/wp amax -> AR1d (off the critical path, overlaps P2)
                w_amax(2, ar1d_i, 0, eng=nc.scalar)
                w_amax(3, ar1d_i, 1, eng=nc.scalar)
                nc.gpsimd.collective_compute(
                    "AllReduce", OP.max, replica_groups=allg,
                    ins=[ar1d_i[:].opt()], outs=[ar1d_o[:].opt()],
                )
                scales_from(slice(3, 5), ar1d_o, slice(0, 2))
                nc.vector.tensor_mul(sxw[:, 2:3], s5[:, 0:1], s5[:, 3:4])
                nc.gpsimd.partition_broadcast(sxwb[:, 2:3], sxw[:, 2:3])

                # ====== P2: q/k projections (d-major); q+k spilled to DRAM
                qa_parts = pcols(16)
                ka_parts = pcols(16)
                with tc.tile_pool(name="kev", bufs=4) as kev_pool, \
                     tc.tile_pool(name="qk_psum", bufs=6, space="PSUM") as qk_psum:
                    for r in range(GROUP):
                        xin = xc0 if r == 0 else x_quant_chunk(r)
                        for wi_, bcol, scol, aparts, spill in (
                            (wq_i, 0, 0, qa_parts, qdeq_sp),
                            (wk_i, 4, 1, ka_parts, kdeq_sp),
                        ):
                            for m in range(HG):
                                ps = qk_psum.tile([128, TLOC], f32, name="ps_qk")
                                for ht in range(16):
                                    nc.tensor.matmul(
                                        ps[:], wi_[:, ht, m * 128 : (m + 1) * 128],
                                        xin[:, ht, :],
                                        start=(ht == 0), stop=(ht == 15),
                                    )
                                kev = kev_pool.tile([128, TLOC], f32, name="kev")
                                nc.scalar.activation(
                                    kev[:], ps[:], AF.Identity,
                                    scale=sxwb[:, scol : scol + 1],
                                    bias=bqk_sb[:, bcol + m : bcol + m + 1],
                                )
                                nc.vector.tensor_reduce(
                                    aparts[:, r * 4 + m : r * 4 + m + 1],
                                    kev[:], AX.X, OP.max,
                                    apply_absolute_value=True,
                                )
                                nc.scalar.dma_start(
                                    out=spill[m * 4 + r], in_=kev[:]
                                )

                # ====== P3: AR2 (q,k amax); wv/wp quant in the AR2 window
                for i, prt in enumerate((qa_parts, ka_parts)):
                    acol = pcols(1)
                    nc.vector.tensor_reduce(acol, prt, AX.X, OP.max)
                    ag_ = p_reduce_max(acol)
                    nc.sync.dma_start(out=ar2_i[:, i : i + 1], in_=ag_[0:1, :])
                nc.gpsimd.collective_compute(
                    "AllReduce", OP.max, replica_groups=allg,
                    ins=[ar2_i[:].opt()], outs=[ar2_o[:].opt()],
                )
                w_quant(2, wv_i)
            wqk_ctx.close()

            # ====== v projection fills the AR2 window (needs no AR2),
            # interleaved with per-head q/k quant so pass-1 h0 starts early
            va_parts = pcols(16)
            with tc.tile_pool(name="rld", bufs=16) as rld_pool, \
                 tc.tile_pool(name="rldq", bufs=16) as rldq_pool, \
                 tc.tile_pool(name="v_psum", bufs=3, space="PSUM") as v_psum, \
                 tc.tile_pool(name="vev", bufs=4) as vev_pool:
                klds = []
                for i in range(16):
                    kld = rld_pool.tile([128, TLOC], f32, name="rld")
                    nc.scalar.dma_start(out=kld[:], in_=kdeq_sp[i])
                    klds.append(kld)
                qlds = []
                for i in range(16):
                    qld = rldq_pool.tile([128, TLOC], f32, name="rldq")
                    nc.scalar.dma_start(out=qld[:], in_=qdeq_sp[i])
                    qlds.append(qld)

                def v_chunk(h):
                    xv = xch_pool.tile([128, 16, TLOC], bf16, name="xc")
                    nc.sync.dma_start(out=xv[:], in_=xint_d[h])
                    for ts in range(4):
                        tsl = h * 4 + ts
                        ps = v_psum.tile([128, 512], f32, name="ps_v")
                        for ht in range(16):
                            nc.tensor.matmul(
                                ps[:], xv[:, ht, ts * 128 : (ts + 1) * 128],
                                wv_i[:, ht, :],
                                start=(ht == 0), stop=(ht == 15),
                            )
                        vtmp = qpool.tile([128, 512], f32, name="vtmp", tag="qs_f32")
                        nc.scalar.activation(
                            vtmp[:], ps[:], AF.Copy, scale=sxwb[:, 2:3]
                        )
                        vev = vev_pool.tile([128, 512], f32, name="vev")
                        nc.vector.tensor_add(vev[:], vtmp[:], bv_sb[:])
                        nc.vector.tensor_reduce(
                            va_parts[:, tsl : tsl + 1], vev[:], AX.X, OP.max,
                            apply_absolute_value=True,
                        )
                        nc.scalar.dma_start(out=vdeq_sp[tsl], in_=vev[:])

                def qk_quant(hm):
                    for r in range(GROUP):
                        sl = slice(r * TLOC, (r + 1) * TLOC)
                        m2 = qpool.tile([128, TLOC], f32, name="km", tag="qs_f32")
                        nc.scalar.activation(
                            m2[:], klds[hm * 4 + r][:], AF.Copy, scale=qf2b[:, 1:2]
                        )
                        rnd(k_int[:, hm, sl], m2[:])
                    for r in range(GROUP):
                        sl = slice(r * TLOC, (r + 1) * TLOC)
                        m = qpool.tile([128, TLOC], f32, name="qm", tag="qs_f32")
                        nc.scalar.activation(
                            m[:], qlds[hm * 4 + r][:], AF.Copy, scale=qf2b[:, 0:1]
                        )
                        rnd(q_int[:, hm, sl], m[:])

                v_chunk(0)
                v_chunk(1)

                g2 = cols(2)[0:1, :]
                nc.sync.dma_start(out=g2, in_=ar2_o[:])
                s_q = cols(1)[0:1, :]
                nc.vector.tensor_scalar(
                    s_q, g2[:, 0:1], Q_MUL / 127.0, 1e-8,
                    op0=OP.mult, op1=OP.max,
                )
                qf = cols(1)[0:1, :]
                nc.vector.reciprocal(qf, s_q)
                nc.vector.tensor_scalar_mul(qf, qf, Q_MUL)
                s_k = cols(1)[0:1, :]
                nc.vector.tensor_scalar(
                    s_k, g2[:, 1:2], 1.0 / 127.0, 1e-8, op0=OP.mult, op1=OP.max
                )
                i_k = cols(1)[0:1, :]
                nc.vector.reciprocal(i_k, s_k)
                s_qk = cols(1)[0:1, :]
                nc.vector.tensor_mul(s_qk, s_q, s_k)
                qf2 = cols(2)[0:1, :]
                nc.vector.tensor_copy(qf2[:, 0:1], qf)
                nc.vector.tensor_copy(qf2[:, 1:2], i_k)
                qf2b = bcast(qf2)
                s_qk_b = bcast(s_qk)
                neg_inv_sqk = cols(1)[0:1, :]
                nc.vector.reciprocal(neg_inv_sqk, s_qk)
                nc.vector.tensor_scalar_mul(neg_inv_sqk, neg_inv_sqk, -1.0)
                nis_b = bcast(neg_inv_sqk)

                qk_quant(0)
                v_chunk(2)
                qk_quant(1)
                v_chunk(3)
                qk_quant(2)
                qk_quant(3)
            xch_ctx.close()
            va = pcols(1)
            nc.vector.tensor_reduce(va, va_parts, AX.X, OP.max)
            vag = p_reduce_max(va)
            nc.sync.dma_start(out=ar3b_i[:], in_=vag[0:1, :])
            nc.gpsimd.collective_compute(
                "AllReduce", OP.max, replica_groups=allg,
                ins=[ar3b_i[:].opt()], outs=[ar3b_o[:].opt()],
            )


            # wp quant + Wp AllGather: off the critical path, after q/k quant
            with tc.tile_pool(name="wvp", bufs=4) as wvp_pool:
                wpb_r = wp_bounce.rearrange("half p hc o -> p half hc o")
                for ht in range(16):
                    wf = wvp_pool.tile([128, 512], f32, name="wvp")
                    nc.sync.dma_start(out=wf[:, :OSL], in_=wp_r[:, ht, :])
                    wm = qpool.tile([128, 512], f32, name="wpm", tag="qs_f32")
                    nc.scalar.activation(
                        wm[:, :OSL], wf[:, :OSL], AF.Copy, scale=i5b[:, 4:5]
                    )
                    wi = qpool.tile([128, OSL], bf16, name="wpi", tag="qs_bf16")
                    rnd(wi[:], wm[:, :OSL])
                    nc.sync.dma_start(
                        out=wpb_r[:, :, ht, :],
                        in_=wi[:].rearrange("p (half o) -> p half o", o=128),
                    )
                nc.gpsimd.collective_compute(
                    "AllGather", OP.bypass, replica_groups=allg,
                    ins=[wp_bounce[:].opt()], outs=[wp_g[:].opt()],
                )

            # ====== P4: pass 1 (stats)
            stats = sbuf.tile([128, 512], f32, name="stats")
            S_all = stats[:, 0:64]
            M_all = stats[:, 64:128]
            with tc.tile_pool(name="p4_psum", bufs=2, space="PSUM") as p4_psum, \
                 tc.tile_pool(name="epool", bufs=2) as e_pool:
                for h in range(HG):
                    for qt in range(16):
                        ps = p4_psum.tile([128, TB], f32, name="ps_a")
                        for jc in range(4):
                            nc.tensor.matmul(
                                ps[:, jc * 512 : (jc + 1) * 512],
                                q_int[:, h, qt * 128 : (qt + 1) * 128],
                                k_int[:, h, jc * 512 : (jc + 1) * 512],
                                start=True, stop=True,
                            )
                        col = h * 16 + qt
                        E = e_pool.tile([128, TB], f32, name="E")
                        nc.scalar.activation(
                            E[:], ps[:], AF.Exp, scale=s_qk_b[:, 0:1],
                            accum_out=S_all[:, col : col + 1],
                        )
                        nc.vector.tensor_scalar(
                            E[:], E[:], 1.0, None, op0=OP.mult,
                            op1=OP.max, accum_out=M_all[:, col : col + 1],
                        )

            # ====== P5a: stats -> AR3a (R)
            Sinv = stats[:, 128:192]
            nc.vector.reciprocal(Sinv, S_all)
            R = stats[:, 192:256]
            nc.vector.tensor_mul(R, M_all, Sinv)
            ra = pcols(1)
            nc.vector.tensor_reduce(ra, R, AX.X, OP.max)
            rag = p_reduce_max(ra)
            nc.sync.dma_start(out=ar3a_i[:], in_=rag[0:1, :])
            nc.gpsimd.collective_compute(
                "AllReduce", OP.max, replica_groups=allg,
                ins=[ar3a_i[:].opt()], outs=[ar3a_o[:].opt()],
            )


        # c' = -ln(S_i)/s_qk, prepared during AR3a.  Computed on [128, 64],
        # xbar-transposed to [64, 128], hi/mid/lo split there, then flattened
        # into the resident cmv_all [3, HG*TB] (row r=(h*16+qt), col p).
        cl = stats[:, 320:384]
        nc.scalar.activation(cl, S_all, AF.Ln)
        cpr = stats[:, 384:448]
        nc.vector.tensor_scalar(cpr, cl, nis_b[:, 0:1], None, op0=OP.mult)
        chib = sbuf.tile([128, 128], bf16, name="chib")
        cmib = sbuf.tile([128, 128], bf16, name="cmib")
        clob = sbuf.tile([128, 128], bf16, name="clob")
        nc.vector.memset(chib[:, 64:], 0.0)
        nc.vector.memset(cmib[:, 64:], 0.0)
        nc.vector.memset(clob[:, 64:], 0.0)
        chif = stats[:, 448:512]
        res1 = stats[:, 128:192]
        res2 = stats[:, 192:256]
        nc.vector.tensor_copy(chib[:, :64], cpr)
        nc.vector.tensor_copy(chif, chib[:, :64])
        nc.vector.tensor_sub(res1, cpr, chif)
        nc.vector.tensor_copy(cmib[:, :64], res1)
        nc.vector.tensor_copy(chif, cmib[:, :64])
        nc.vector.tensor_sub(res2, res1, chif)
        nc.vector.tensor_copy(clob[:, :64], res2)
        # xbar-transpose each bf16 split to [64(h qt), 128(p)], then flatten
        # rows into the resident cmv_all [3, HG*TB] (index h*2048+qt*128+p)
        cT0 = sbuf.tile([128, 128], bf16, name="cT0")
        cT1 = sbuf.tile([128, 128], bf16, name="cT1")
        cT2 = sbuf.tile([128, 128], bf16, name="cT2")
        cmv_all = sbuf.tile([3, HG * TB], bf16, name="cmv_all")
        for ci, t, ct in ((0, chib, cT0), (1, cmib, cT1), (2, clob, cT2)):
            nc.sync.dma_start_transpose(out=ct[:], in_=t[:])
            nc.sync.dma_start(
                out=cmv_all[ci : ci + 1, :].rearrange("o (r c) -> o r c", c=128),
                in_=ct[0:64, :],
            )

        gA = cols(1)[0:1, :]
        nc.sync.dma_start(out=gA, in_=ar3a_o[:])
        s_attn = cols(1)[0:1, :]
        nc.vector.tensor_scalar(
            s_attn, gA, 1.0 / 127.0, 1e-8, op0=OP.mult, op1=OP.max
        )
        lnsa = cols(1)[0:1, :]
        nc.scalar.activation(lnsa, s_attn, AF.Ln)
        nc.vector.tensor_scalar_mul(lnsa, lnsa, -1.0)
        eb_b = bcast(lnsa)
        gV = cols(1)[0:1, :]
        nc.sync.dma_start(out=gV, in_=ar3b_o[:])
        s_v = cols(1)[0:1, :]
        nc.vector.tensor_scalar(s_v, gV, 1.0 / 127.0, 1e-8, op0=OP.mult, op1=OP.max)
        i_v = cols(1)[0:1, :]
        nc.vector.reciprocal(i_v, s_v)
        i_v_b = bcast(i_v)
        s_av = cols(1)[0:1, :]
        nc.vector.tensor_mul(s_av, s_attn, s_v)
        s_av_b = bcast(s_av)

        # ====== P6: pass 2 + S@V (v quantized from spill after AR3b)
        out_T, out_T_free = tc.tile([128, HG, TB], f32, name="out_T")
        oa_parts = pcols(16)
        vq_done = [False]
        with tc.tile_pool(name="p6_psum", bufs=3, space="PSUM") as p6_psum, \
             tc.tile_pool(name="sv_psum", bufs=2, space="PSUM") as sv_psum, \
             tc.tile_pool(name="pint", bufs=2) as pint_pool, \
             tc.tile_pool(name="ps_scr", bufs=3) as ps_scr, \
             tc.tile_pool(name="vld", bufs=3) as vld_pool:
            for h in range(HG):
                cmv = cmv_all[:, h * TB : (h + 1) * TB]
                for qcp in range(2):
                    p_int = pint_pool.tile([128, 16, 1024], bf16, name="p_int")
                    for jt in range(16):
                        ps2 = p6_psum.tile([128, 1024], f32, name="ps2")
                        for half in range(2):
                            q0 = qcp * 1024 + half * 512
                            nc.tensor.matmul(
                                ps2[:, half * 512 : (half + 1) * 512],
                                k_int[:, h, jt * 128 : (jt + 1) * 128],
                                q_int[:, h, q0 : q0 + 512],
                                start=True, stop=False,
                            )
                        for half in range(2):
                            q0 = qcp * 1024 + half * 512
                            nc.tensor.matmul(
                                ps2[:, half * 512 : (half + 1) * 512],
                                ones3[:], cmv[:, q0 : q0 + 512],
                                start=False, stop=True, skip_group_check=True,
                            )
                        PS = ps_scr.tile([128, 1024], f32, name="PS")
                        nc.scalar.activation(
                            PS[:], ps2[:], AF.Exp,
                            scale=s_qk_b[:, 0:1], bias=eb_b[:, 0:1],
                        )
                        rnd(p_int[:, jt, :], PS[:])
                    if not vq_done[0]:
                        # quantize v now (AR3b has landed); needed by first S@V
                        for tsl in range(16):
                            vld = vld_pool.tile([128, 512], f32, name="vld")
                            nc.scalar.dma_start(out=vld[:], in_=vdeq_sp[tsl])
                            m = qpool.tile([128, 512], f32, name="vm", tag="qs_f32")
                            nc.scalar.activation(
                                m[:], vld[:], AF.Copy, scale=i_v_b[:, 0:1]
                            )
                            rnd(v_int[:, tsl, :], m[:])
                        vq_done[0] = True
                    for qch in range(2):
                        q0 = qcp * 1024 + qch * 512
                        ps3 = sv_psum.tile([128, 512], f32, name="ps3")
                        for jt in range(16):
                            nc.tensor.matmul(
                                ps3[:], v_int[:, jt, h * 128 : (h + 1) * 128],
                                p_int[:, jt, qch * 512 : (qch + 1) * 512],
                                start=(jt == 0), stop=(jt == 15),
                            )
                        nc.scalar.activation(
                            out_T[:, h, q0 : q0 + 512], ps3[:], AF.Copy,
                            scale=s_av_b[:, 0:1],
                        )
                        oi = h * 4 + qcp * 2 + qch
                        nc.vector.tensor_reduce(
                            oa_parts[:, oi : oi + 1],
                            out_T[:, h, q0 : q0 + 512], AX.X, OP.max,
                            apply_absolute_value=True,
                        )

        # ====== P7: out amax -> AR4 -> masked quantize -> A2A
        oc_ = pcols(1)
        nc.vector.tensor_reduce(oc_, oa_parts, AX.X, OP.max)
        ocg = p_reduce_max(oc_)
        nc.sync.dma_start(out=ar4_i[:], in_=ocg[0:1, :])
        nc.gpsimd.collective_compute(
            "AllReduce", OP.max, replica_groups=allg,
            ins=[ar4_i[:].opt()], outs=[ar4_o[:].opt()],
        )
        gO = cols(1)[0:1, :]
        nc.sync.dma_start(out=gO, in_=ar4_o[:])
        s_out = cols(1)[0:1, :]
        nc.vector.tensor_scalar(s_out, gO, 1.0 / 127.0, 1e-8, op0=OP.mult, op1=OP.max)
        i_out = cols(1)[0:1, :]
        nc.vector.reciprocal(i_out, s_out)
        s_op = cols(1)[0:1, :]
        nc.vector.tensor_mul(s_op, s_out, s5[:, 4:5])
        s_op_b = bcast(s_op)
        iobm = cols(2)[0:1, :]
        nc.vector.tensor_mul(iobm[:, 0:1], i_out, bm_sb[:, 0:1])
        nc.vector.tensor_mul(iobm[:, 1:2], i_out, bm_sb[:, 1:2])
        iobm_b = bcast(iobm)

        # quantize out into the 8 A2A shards: shard j = token chunk j%4,
        # scaled by bmask[j//4] (exact zeros for the other batch's shards)
        with tc.tile_pool(name="om2", bufs=2) as om2_pool, \
             tc.tile_pool(name="a2ast", bufs=3) as a2ast_pool:
            for j in range(NC):
                ch = j % 4
                csl = slice(ch * TLOC, (ch + 1) * TLOC)
                m = om2_pool.tile([128, HG, TLOC], f32, name="om")
                nc.scalar.activation(
                    m[:], out_T[:, :, csl], AF.Copy,
                    scale=iobm_b[:, j // 4 : j // 4 + 1],
                )
                st = a2ast_pool.tile([128, HG, TLOC], bf16, name="a2st")
                rnd(st[:], m[:])
                nc.scalar.dma_start(out=a2a_in[j], in_=st[:])
        nc.gpsimd.collective_compute(
            "AllToAll", OP.bypass, replica_groups=allg,
            ins=[a2a_in[:].opt()], outs=[a2a_out[:].opt()],
        )
        out_T_free()

        # ====== P8: output projection
        out_r = out_ext.rearrange("(ot p) t -> p ot t", p=128)
        a2a_lo = a2a_out[0:GROUP].rearrange("s p h t -> p s h t")
        a2a_hi = a2a_out[GROUP:NC].rearrange("s p h t -> p s h t")
        with tc.tile_pool(name="p8_psum", bufs=4, space="PSUM") as p8_psum, \
             tc.tile_pool(name="wcol8", bufs=3) as wcol_pool, \
             tc.tile_pool(name="fin", bufs=3) as fin_pool, \
             tc.tile_pool(name="oin", bufs=1) as oin_pool:
            oin_a = oin_pool.tile([128, GROUP, HG, TLOC], bf16, name="oin_a")
            nc.sync.dma_start(out=oin_a[:], in_=a2a_lo)
            oin_b = oin_pool.tile([128, GROUP, HG, TLOC], bf16, name="oin_b")
            nc.sync.dma_start(out=oin_b[:], in_=a2a_hi)
            oin4 = oin_pool.tile([128, GROUP, HG, TLOC], bf16, name="oin")
            nc.vector.tensor_add(oin4[:], oin_a[:], oin_b[:])
            oin = oin4.rearrange("p s h t -> p (s h) t")
            for ot in range(16):
                wcol = wcol_pool.tile([128, 16, 128], bf16, name="wcol")
                nc.sync.dma_start(out=wcol[:], in_=wp_g[ot // 2, ot % 2])
                ps = p8_psum.tile([128, TLOC], f32, name="ps_p")
                for hc in range(16):
                    nc.tensor.matmul(
                        ps[:], wcol[:, hc, :], oin[:, hc, :],
                        start=(hc == 0), stop=(hc == 15),
                    )
                fin = fin_pool.tile([128, TLOC], f32, name="fin")
                nc.scalar.activation(
                    fin[:], ps[:], AF.Identity,
                    scale=s_op_b[:, 0:1], bias=bp_sb[:, ot : ot + 1],
                )
                nc.sync.dma_start(out=out_r[:, ot, :], in_=fin[:])

        v_int_free()
        k_int_free()
        q_int_free()

    nc.compile()
    return nc


def _get_compiled():
    if "nc" not in _COMPILED:
        _COMPILED["nc"] = _build()
    return _COMPILED["nc"]


def make_in_maps(hidden_states, Wq, bq, Wk, bk, Wv, bv, Wp, bp):
    hs = np.asarray(hidden_states, dtype=np.float32)
    wT = [
        np.ascontiguousarray(np.asarray(W, np.float32).T)
        for W in (Wq, Wk, Wv, Wp)
    ]
    bq_ = np.asarray(bq, np.float32)
    bk_ = np.asarray(bk, np.float32)
    bv_ = np.asarray(bv, np.float32)
    bp_t = np.ascontiguousarray(np.asarray(bp, np.float32).reshape(16, 128).T)
    xbT = [np.ascontiguousarray(hs[b].T) for b in range(B)]
    in_maps = []
    for c in range(NC):
        b = c // GROUP
        g = c % GROUP
        x_Tc = np.ascontiguousarray(hs[b, g * TLOC : (g + 1) * TLOC, :].T)
        dsl = slice(512 * g, 512 * (g + 1))
        wqkv_c = np.ascontiguousarray(
            np.stack([wT[0][:, dsl], wT[1][:, dsl], wT[2][:, dsl]], axis=0)
        )
        wp_c = np.ascontiguousarray(wT[3][:, c * OSL : (c + 1) * OSL])
        bqk_c = np.ascontiguousarray(
            np.concatenate(
                [bq_[dsl].reshape(4, 128).T, bk_[dsl].reshape(4, 128).T], axis=1
            )
        )
        bv_c = np.ascontiguousarray(
            np.broadcast_to(bv_[dsl][None, :], (128, 512))
        )
        bm = np.zeros((1, 2), np.float32)
        bm[0, b] = 1.0
        in_maps.append(
            {"x_T": x_Tc, "x_b_T": xbT[b], "wqkv": wqkv_c, "wp_sl": wp_c,
             "bqk": bqk_c, "bv_b": bv_c, "bp_t": bp_t, "bmask": bm}
        )
    return in_maps


def kernel(hidden_states, Wq, bq, Wk, bk, Wv, bv, Wp, bp):
    from concourse.bass_utils import run_bass_kernel_spmd

    trace = bool(int(os.environ.get("KERNEL_TRACE", "0")))
    nc = _get_compiled()
    in_maps = make_in_maps(hidden_states, Wq, bq, Wk, bk, Wv, bv, Wp, bp)
    res = run_bass_kernel_spmd(nc, in_maps, core_ids=list(range(NC)), trace=trace)
    kernel.last_exec_time_ns = res.exec_time_ns
    kernel.last_results = res.results

    out = np.empty((B, S, H), dtype=np.float32)
    for c in range(NC):
        b = c // GROUP
        t0 = (c % GROUP) * TLOC
        out[b, t0 : t0 + TLOC, :] = res.results[c]["out"].T
    return out


kernel.last_exec_time_ns = None
kernel.last_results = None
